# revision 22
# baseline (speedup 1.0000x reference)
"""Trainium2 Bass kernel for causal multi-head attention (B=4, T=2048, D=1024, H=16).

Sharding: 8 cores = 4 batches x 2 head-groups (8 heads each).
Per core pipeline (Tile framework, single SPMD program):
  proj(j): Q/K projections into transposed per-head-pair layout QT/KT [128=2*64, 512],
           V projection into [t, 8*65] layout (65th col per head = ones, for rowsums)
  attn(j): per (q-range of 512, head-pair): causal flash attention in transposed
           layout: ST[k,q] = KT-slice^T @ QT-slice, additive -50 mask on the
           128-wide diagonal slab (DVE), PT = exp(ST) (ACT),
           OT[hd+1, q] += [V|1]^T @ PT (bf16).
           Rowsums batched per j into [8,512]; one fast reciprocal; per-head
           partition-broadcast + multiply normalizes.
  outproj(j): YT[dout, q] = Wo_g^T @ OT + bias (g=0 adds bo), bf16 partial
           written straight to DRAM.  NO collective: the two cores of a batch
           each emit a full-[1024, T] partial; the host sums the pair.
Program order: proj(0); for j: attn(j); proj(j+1); outproj(j) — the normalize
chain of attn(j) hides behind proj(j+1) tensor work.
Host: transpose/slice weights, sum per-core partials, assemble [B, T, D].
"""

import numpy as np

B, T, D, H, HD = 4, 2048, 1024, 16, 64
NCORES = 8
NP = 4          # head pairs per core
NJ = 4          # q-ranges of 512
QW = 512
TB = T // 128   # 16

_CACHE = {}


def _build_nc():
    import concourse.mybir as mybir
    import concourse.tile as tile
    from concourse import bacc

    F32 = mybir.dt.float32
    BF16 = mybir.dt.bfloat16
    AF = mybir.ActivationFunctionType

    nc = bacc.Bacc(None, target_bir_lowering=False)
    xt_d = nc.declare_dram_parameter("xt", [D, T], BF16, isOutput=False)
    wq_d = nc.declare_dram_parameter("wq", [D, 512], BF16, isOutput=False)
    wk_d = nc.declare_dram_parameter("wk", [D, 512], BF16, isOutput=False)
    wv_d = nc.declare_dram_parameter("wv", [D, 512], BF16, isOutput=False)
    wo_d = nc.declare_dram_parameter("wo", [512, D], BF16, isOutput=False)
    bias_d = nc.declare_dram_parameter("bias", [128, 8], F32, isOutput=False)
    mask_d = nc.declare_dram_parameter("mask", [128, 256], BF16, isOutput=False)
    yt_d = nc.declare_dram_parameter("yt", [D, T], BF16, isOutput=True)

    with tile.TileContext(nc) as tc:
        with (
            tc.tile_pool(name="persist", bufs=1) as pers,
            tc.tile_pool(name="work", bufs=1) as work,
            tc.tile_pool(name="dram", bufs=1, space="DRAM") as dram,
            tc.tile_pool(name="psum", bufs=1, space="PSUM") as psum,
        ):
            rsT8 = pers.tile([128, 8], F32)
            rec8 = pers.tile([128, 8], F32)
            rsum_d = dram.tile([NP, 1024], F32)
            rec_d = dram.tile([NP, 1024], F32)
            # per-p reciprocal rows, all on partition 0 (hw partition_broadcast
            # does not honor input partition offsets)
            recrow4 = pers.tile([1, NP * 1024], F32)
            qt = pers.tile([128, NP, T], BF16)
            kt = pers.tile([128, NP, T], BF16)
            v = pers.tile([128, TB, 8 * 65], BF16)
            ot = pers.tile([128, NP, T], BF16)
            mneg = pers.tile([128, 256], BF16)
            wo = pers.tile([128, 4, D], BF16)
            bias = pers.tile([128, 8], F32)
            wq = pers.tile([128, 8, 512], BF16)
            wk = pers.tile([128, 8, 512], BF16)
            wv = pers.tile([128, 8, 512], BF16)
            xsb = pers.tile([128, 8, NJ, QW], BF16)

            # DMA priority order: operands of the first matmuls first.
            for c in range(8):
                nc.sync.dma_start(
                    out=xsb[:, c, 0, :], in_=xt_d[c * 128:(c + 1) * 128, 0:QW]
                )
                nc.sync.dma_start(out=wq[:, c, :], in_=wq_d[c * 128:(c + 1) * 128, :])
            for c in range(8):
                nc.sync.dma_start(out=wk[:, c, :], in_=wk_d[c * 128:(c + 1) * 128, :])
            for c in range(8):
                nc.sync.dma_start(out=wv[:, c, :], in_=wv_d[c * 128:(c + 1) * 128, :])
            for j in range(1, NJ):
                for c in range(8):
                    nc.sync.dma_start(
                        out=xsb[:, c, j, :],
                        in_=xt_d[c * 128:(c + 1) * 128, j * QW:(j + 1) * QW],
                    )
            nc.sync.dma_start(out=mneg[:], in_=mask_d[:])
            nc.sync.dma_start(out=bias[:], in_=bias_d[:])
            nc.sync.dma_start(out=wo[:], in_=wo_d.rearrange("(c p) n -> p c n", p=128))

            def proj(j):
                jr = slice(j * QW, (j + 1) * QW)
                for p in range(NP):
                    proj_qk(j, p)
                for sub in range(4):
                    proj_v(j, sub)

            def proj_qk(j, p):
                jr = slice(j * QW, (j + 1) * QW)
                for w_sb, dst in ((wq, qt), (wk, kt)):
                    acc = psum.tile([128, QW], F32, tag="small", bufs=2)
                    for c in range(8):
                        nc.tensor.matmul(
                            acc[:],
                            w_sb[:, c, p * 128:(p + 1) * 128],
                            xsb[:, c, j, :],
                            start=(c == 0),
                            stop=(c == 7),
                        )
                    nc.vector.tensor_copy(dst[:, p, jr], acc[:])

            def proj_v(j, sub):
                i = 4 * j + sub
                acc = psum.tile([128, QW], F32, tag="small", bufs=2)
                for c in range(8):
                    nc.tensor.matmul(
                        acc[:],
                        xsb[:, c, j, sub * 128:(sub + 1) * 128],
                        wv[:, c, :],
                        start=(c == 0),
                        stop=(c == 7),
                    )
                vblk = v[:, i, :].rearrange("p (h c) -> p h c", c=65)
                nc.vector.tensor_copy(
                    vblk[:, :, 0:64],
                    acc[:].rearrange("p (h c) -> p h c", c=64),
                )
                nc.gpsimd.memset(vblk[:, :, 64:65], 1.0)

            def attn_p(j, p, last=False):
                jr = slice(j * QW, (j + 1) * QW)
                if True:
                    hA, hB = 2 * p, 2 * p + 1
                    o_A = psum.tile([65, QW], F32, tag="o", bufs=2)
                    o_B = psum.tile([65, QW], F32, tag="o", bufs=2)
                    nkb = 4 * j + 4
                    for kb in range(nkb):
                        o = kb - 4 * j  # diagonal offset; < 0 means full block
                        lo = 128 * o if o > 0 else 0  # first live q col in range
                        st = psum.tile([128, 1024], F32, tag="st", bufs=2)
                        kcols = slice(kb * 128, (kb + 1) * 128)
                        qcols = slice(j * QW + lo, (j + 1) * QW)
                        nc.tensor.matmul(
                            st[:, lo:QW],
                            kt[0:64, p, kcols],
                            qt[0:64, p, qcols],
                            start=True, stop=True, tile_position=(0, 0),
                        )
                        nc.tensor.matmul(
                            st[:, QW + lo:2 * QW],
                            kt[64:128, p, kcols],
                            qt[64:128, p, qcols],
                            start=True, stop=True, tile_position=(64, 0),
                        )
                        stv = st[:].rearrange("p (h q) -> p h q", h=2)
                        if o >= 0:
                            # additive causal mask on the diagonal 128-slab
                            nc.vector.tensor_add(
                                stv[:, :, lo:lo + 128],
                                stv[:, :, lo:lo + 128],
                                mneg[:].rearrange("p (h q) -> p h q", h=2),
                            )
                        pt = work.tile([128, 1024], BF16, tag="pt", bufs=3)
                        nc.scalar.activation(
                            pt[:].rearrange("p (h q) -> p h q", h=2)[:, :, lo:QW],
                            stv[:, :, lo:QW],
                            AF.Exp,
                        )
                        nc.tensor.matmul(
                            o_A[:, lo:QW],
                            v[:, kb, hA * 65:(hA + 1) * 65],
                            pt[:, lo:QW],
                            start=(kb == 0), stop=(kb == nkb - 1),
                        )
                        nc.tensor.matmul(
                            o_B[:, lo:QW],
                            v[:, kb, hB * 65:(hB + 1) * 65],
                            pt[:, QW + lo:2 * QW],
                            start=(kb == 0), stop=(kb == nkb - 1),
                        )
                    # copy psum accumulators out so the o slots free early
                    ocp = work.tile([65, 1024], F32, tag="ocp", bufs=3)
                    nc.vector.tensor_copy(ocp[:, 0:QW], o_A[:])
                    nc.vector.tensor_copy(ocp[:, QW:1024], o_B[:])
                    # rowsum reciprocal: a [1,1024] DVE reciprocal is serial on
                    # one lane (~6.5us).  Instead bounce the row through DRAM
                    # into [128,8] partition-major layout (DRAM APs are linear
                    # so the transpose balances), reciprocal there (free size
                    # 8, ~0.2us), and bounce back to row layout for the
                    # broadcast.  No tensor-engine involvement.
                    # For the very last head-pair there is no later tensor work
                    # to hide the 4-hop DMA latency behind, so the direct
                    # (slow but short-chain) reciprocal wins there.
                    if last:
                        for s in range(2):
                            nc.vector.reciprocal(
                                recrow4[0:1, 1024 * p + s * QW:1024 * p + (s + 1) * QW],
                                ocp[64:65, s * QW:(s + 1) * QW],
                            )
                    else:
                        nc.sync.dma_start(out=rsum_d[p:p + 1, :], in_=ocp[64:65, :])
                        nc.sync.dma_start(
                            out=rsT8[:],
                            in_=rsum_d[p:p + 1, :].rearrange("p (c q) -> p q c", c=8),
                        )
                        nc.vector.reciprocal(rec8[:], rsT8[:])
                        nc.sync.dma_start(out=rec_d[p:p + 1, :], in_=rec8[:])
                        nc.sync.dma_start(
                            out=recrow4[0:1, 1024 * p:1024 * (p + 1)],
                            in_=rec_d[p:p + 1, :].rearrange("p (q c) -> p c q", c=8),
                        )
                    for s in range(2):
                        bc = work.tile([64, QW], F32, tag="bc", bufs=3)
                        nc.gpsimd.partition_broadcast(
                            bc[:],
                            recrow4[0:1, 1024 * p + s * QW:1024 * p + (s + 1) * QW],
                            channels=64,
                        )
                        nc.vector.tensor_mul(
                            ot[s * 64:(s + 1) * 64, p, jr],
                            ocp[0:64, s * QW:(s + 1) * QW],
                            bc[:],
                        )

            def outproj(j):
                jr = slice(j * QW, (j + 1) * QW)
                for n in range(8):
                    yps = psum.tile([128, QW], F32, tag="small", bufs=2)
                    for c in range(4):
                        nc.tensor.matmul(
                            yps[:],
                            wo[:, c, n * 128:(n + 1) * 128],
                            ot[:, c, jr],
                            start=(c == 0), stop=(c == 3),
                        )
                    ysb = work.tile([128, QW], BF16, tag="ysb", bufs=3)
                    nc.vector.tensor_scalar_add(ysb[:], yps[:], bias[:, n:n + 1])
                    nc.sync.dma_start(
                        out=yt_d[n * 128:(n + 1) * 128, jr], in_=ysb[:]
                    )

            proj(0)
            for j in range(NJ):
                # interleave next j-range's projections between attention
                # head-pairs so the tensor engine fills exp-paced gaps
                for p in range(NP):
                    attn_p(j, p, last=(j == NJ - 1 and p == NP - 1))
                    if j + 1 < NJ:
                        proj_qk(j + 1, p)
                if j + 1 < NJ:
                    for sub in range(4):
                        proj_v(j + 1, sub)
                outproj(j)

    nc.finalize()
    return nc


def _prep_inputs(x, Wq, Wk, Wv, Wo, bo):
    """Build the 8 per-core input maps (host-side layout prep only)."""
    import ml_dtypes

    scale = 1.0 / np.sqrt(np.float32(HD))
    kr = np.arange(128, dtype=np.float32)[:, None]
    qc = np.arange(128, dtype=np.float32)[None, :]
    tri = np.where(qc >= kr, np.float32(0.0), np.float32(-50.0))
    mneg = np.tile(tri, (1, 2)).astype(ml_dtypes.bfloat16)

    in_maps = []
    for c in range(NCORES):
        b, g = c // 2, c % 2
        hs = slice(g * 8, (g + 1) * 8)
        xt = np.ascontiguousarray(x[b].T).astype(ml_dtypes.bfloat16)
        wq = np.ascontiguousarray(Wq[hs].reshape(512, D).T * scale).astype(ml_dtypes.bfloat16)
        wk = np.ascontiguousarray(Wk[hs].reshape(512, D).T).astype(ml_dtypes.bfloat16)
        wv = np.ascontiguousarray(Wv[hs].reshape(512, D).T).astype(ml_dtypes.bfloat16)
        wo = np.ascontiguousarray(Wo[:, g * 512:(g + 1) * 512].T).astype(ml_dtypes.bfloat16)
        if g == 0:
            bias = np.ascontiguousarray(bo.reshape(8, 128).T)
        else:
            bias = np.zeros((128, 8), np.float32)
        in_maps.append(
            {"xt": xt, "wq": wq, "wk": wk, "wv": wv, "wo": wo, "bias": bias, "mask": mneg}
        )
    return in_maps


def _run(inputs, trace=False, trace_cores=None):
    from concourse.bass_utils import run_bass_kernel_spmd

    if "nc" not in _CACHE:
        _CACHE["nc"] = _build_nc()
    nc = _CACHE["nc"]
    in_maps = _prep_inputs(
        inputs["x"], inputs["Wq"], inputs["Wk"], inputs["Wv"], inputs["Wo"], inputs["bo"]
    )
    r = run_bass_kernel_spmd(
        nc, in_maps, list(range(NCORES)), trace=trace, trace_cores=trace_cores
    )
    y = np.empty((B, T, D), np.float32)
    for b in range(B):
        yt = np.asarray(r.results[2 * b]["yt"], dtype=np.float32) + np.asarray(
            r.results[2 * b + 1]["yt"], dtype=np.float32
        )
        y[b] = yt.T
    return y, r


def kernel(**inputs):
    y, _ = _run(inputs, trace=False)
    return y


# revision 26
# speedup vs baseline: 1.0027x; 1.0027x over previous
"""Trainium2 Bass kernel for causal multi-head attention (B=4, T=2048, D=1024, H=16).

Sharding: 8 cores = 4 batches x 2 head-groups (8 heads each).
Per core pipeline (Tile framework, single SPMD program):
  proj(j): Q/K projections into transposed per-head-pair layout QT/KT [128=2*64, 512],
           V projection into [t, 8*65] layout (65th col per head = ones, for rowsums)
  attn(j): per (q-range of 512, head-pair): causal flash attention in transposed
           layout: ST[k,q] = KT-slice^T @ QT-slice, additive -50 mask on the
           128-wide diagonal slab (DVE), PT = exp(ST) (ACT),
           OT[hd+1, q] += [V|1]^T @ PT (bf16).
           Rowsums batched per j into [8,512]; one fast reciprocal; per-head
           partition-broadcast + multiply normalizes.
  outproj(j): YT[dout, q] = Wo_g^T @ OT + bias (g=0 adds bo), bf16 partial
           written straight to DRAM.  NO collective: the two cores of a batch
           each emit a full-[1024, T] partial; the host sums the pair.
Program order: proj(0); for j: attn(j); proj(j+1); outproj(j) — the normalize
chain of attn(j) hides behind proj(j+1) tensor work.
Host: transpose/slice weights, sum per-core partials, assemble [B, T, D].
"""

import numpy as np

B, T, D, H, HD = 4, 2048, 1024, 16, 64
NCORES = 8
NP = 4          # head pairs per core
NJ = 4          # q-ranges of 512
QW = 512
TB = T // 128   # 16

_CACHE = {}


def _build_nc():
    import concourse.mybir as mybir
    import concourse.tile as tile
    from concourse import bacc

    F32 = mybir.dt.float32
    BF16 = mybir.dt.bfloat16
    AF = mybir.ActivationFunctionType

    nc = bacc.Bacc(None, target_bir_lowering=False)
    xt_d = nc.declare_dram_parameter("xt", [D, T], BF16, isOutput=False)
    wq_d = nc.declare_dram_parameter("wq", [D, 512], BF16, isOutput=False)
    wk_d = nc.declare_dram_parameter("wk", [D, 512], BF16, isOutput=False)
    wv_d = nc.declare_dram_parameter("wv", [D, 512], BF16, isOutput=False)
    wo_d = nc.declare_dram_parameter("wo", [512, D], BF16, isOutput=False)
    bias_d = nc.declare_dram_parameter("bias", [128, 8], F32, isOutput=False)
    mask_d = nc.declare_dram_parameter("mask", [128, 256], BF16, isOutput=False)
    yt_d = nc.declare_dram_parameter("yt", [D, T], BF16, isOutput=True)

    with tile.TileContext(nc) as tc:
        with (
            tc.tile_pool(name="persist", bufs=1) as pers,
            tc.tile_pool(name="work", bufs=1) as work,
            tc.tile_pool(name="dram", bufs=1, space="DRAM") as dram,
            tc.tile_pool(name="psum", bufs=1, space="PSUM") as psum,
        ):
            rsT8 = pers.tile([128, 8], F32)
            rec8 = pers.tile([128, 8], F32)
            rsum_d = dram.tile([NP, 1024], F32)
            rec_d = dram.tile([NP, 1024], F32)
            # per-p reciprocal rows, all on partition 0 (hw partition_broadcast
            # does not honor input partition offsets)
            recrow4 = pers.tile([1, NP * 1024], F32)
            qt = pers.tile([128, NP, T], BF16)
            kt = pers.tile([128, NP, T], BF16)
            v = pers.tile([128, TB, 8 * 65], BF16)
            ot = pers.tile([128, NP, T], BF16)
            mneg = pers.tile([128, 256], BF16)
            wo = pers.tile([128, 4, D], BF16)
            bias = pers.tile([128, 8], F32)
            wq = pers.tile([128, 8, 512], BF16)
            wk = pers.tile([128, 8, 512], BF16)
            wv = pers.tile([128, 8, 512], BF16)
            xsb = pers.tile([128, 8, NJ, QW], BF16)

            # DMA priority order: operands of the first matmuls first.
            for c in range(8):
                nc.sync.dma_start(
                    out=xsb[:, c, 0, :], in_=xt_d[c * 128:(c + 1) * 128, 0:QW]
                )
                nc.sync.dma_start(out=wq[:, c, :], in_=wq_d[c * 128:(c + 1) * 128, :])
            for c in range(8):
                nc.sync.dma_start(out=wk[:, c, :], in_=wk_d[c * 128:(c + 1) * 128, :])
            for c in range(8):
                nc.sync.dma_start(out=wv[:, c, :], in_=wv_d[c * 128:(c + 1) * 128, :])
            for j in range(1, NJ):
                for c in range(8):
                    nc.sync.dma_start(
                        out=xsb[:, c, j, :],
                        in_=xt_d[c * 128:(c + 1) * 128, j * QW:(j + 1) * QW],
                    )
            nc.sync.dma_start(out=mneg[:], in_=mask_d[:])
            nc.sync.dma_start(out=bias[:], in_=bias_d[:])
            nc.sync.dma_start(out=wo[:], in_=wo_d.rearrange("(c p) n -> p c n", p=128))
            # pre-warm the Exp activation table so the first real exp in the
            # attention loop doesn't pay the table load
            nc.scalar.activation(recrow4[0:1, 0:1], xsb[0:1, 0, 0, 0:1], AF.Exp)

            # Filler units: projection / output-projection work chopped into
            # single-instruction generator steps, drained one step at a time
            # into the tensor-idle bubbles of the attention block loop.
            def qk_gen(j, p):
                jr = slice(j * QW, (j + 1) * QW)
                for w_sb, dst in ((wq, qt), (wk, kt)):
                    acc = psum.tile([128, QW], F32, tag="small", bufs=2)
                    for c in range(8):
                        nc.tensor.matmul(
                            acc[:],
                            w_sb[:, c, p * 128:(p + 1) * 128],
                            xsb[:, c, j, :],
                            start=(c == 0),
                            stop=(c == 7),
                        )
                        yield
                    nc.vector.tensor_copy(dst[:, p, jr], acc[:])
                    yield

            def v_gen(j, sub):
                i = 4 * j + sub
                acc = psum.tile([128, QW], F32, tag="small", bufs=2)
                for c in range(8):
                    nc.tensor.matmul(
                        acc[:],
                        xsb[:, c, j, sub * 128:(sub + 1) * 128],
                        wv[:, c, :],
                        start=(c == 0),
                        stop=(c == 7),
                    )
                    yield
                vblk = v[:, i, :].rearrange("p (h c) -> p h c", c=65)
                nc.vector.tensor_copy(
                    vblk[:, :, 0:64],
                    acc[:].rearrange("p (h c) -> p h c", c=64),
                )
                nc.gpsimd.memset(vblk[:, :, 64:65], 1.0)
                yield

            pending = []

            def drain(n):
                while n > 0 and pending:
                    try:
                        next(pending[0])
                        n -= 1
                    except StopIteration:
                        pending.pop(0)

            def drain_all():
                while pending:
                    try:
                        next(pending[0])
                    except StopIteration:
                        pending.pop(0)

            def proj(j):
                pending.extend([qk_gen(j, p) for p in range(NP)])
                pending.extend([v_gen(j, sub) for sub in range(4)])
                drain_all()

            def attn_p(j, p, last=False):
                jr = slice(j * QW, (j + 1) * QW)
                if True:
                    hA, hB = 2 * p, 2 * p + 1
                    o_A = psum.tile([65, QW], F32, tag="o", bufs=2)
                    o_B = psum.tile([65, QW], F32, tag="o", bufs=2)
                    nkb = 4 * j + 4
                    for kb in range(nkb):
                        o = kb - 4 * j  # diagonal offset; < 0 means full block
                        lo = 128 * o if o > 0 else 0  # first live q col in range
                        st = psum.tile([128, 1024], F32, tag="st", bufs=2)
                        kcols = slice(kb * 128, (kb + 1) * 128)
                        qcols = slice(j * QW + lo, (j + 1) * QW)
                        nc.tensor.matmul(
                            st[:, lo:QW],
                            kt[0:64, p, kcols],
                            qt[0:64, p, qcols],
                            start=True, stop=True, tile_position=(0, 0),
                        )
                        nc.tensor.matmul(
                            st[:, QW + lo:2 * QW],
                            kt[64:128, p, kcols],
                            qt[64:128, p, qcols],
                            start=True, stop=True, tile_position=(64, 0),
                        )
                        stv = st[:].rearrange("p (h q) -> p h q", h=2)
                        if o >= 0:
                            # additive causal mask on the diagonal 128-slab
                            nc.vector.tensor_add(
                                stv[:, :, lo:lo + 128],
                                stv[:, :, lo:lo + 128],
                                mneg[:].rearrange("p (h q) -> p h q", h=2),
                            )
                        pt = work.tile([128, 1024], BF16, tag="pt", bufs=3)
                        nc.scalar.activation(
                            pt[:].rearrange("p (h q) -> p h q", h=2)[:, :, lo:QW],
                            stv[:, :, lo:QW],
                            AF.Exp,
                        )
                        drain(1)  # fill the exp-wait bubble with a filler matmul
                        nc.tensor.matmul(
                            o_A[:, lo:QW],
                            v[:, kb, hA * 65:(hA + 1) * 65],
                            pt[:, lo:QW],
                            start=(kb == 0), stop=(kb == nkb - 1),
                        )
                        nc.tensor.matmul(
                            o_B[:, lo:QW],
                            v[:, kb, hB * 65:(hB + 1) * 65],
                            pt[:, QW + lo:2 * QW],
                            start=(kb == 0), stop=(kb == nkb - 1),
                        )
                    # copy psum accumulators out so the o slots free early
                    ocp = work.tile([65, 1024], F32, tag="ocp", bufs=3)
                    nc.vector.tensor_copy(ocp[:, 0:QW], o_A[:])
                    nc.vector.tensor_copy(ocp[:, QW:1024], o_B[:])
                    # rowsum reciprocal: a [1,1024] DVE reciprocal is serial on
                    # one lane (~6.5us).  Instead bounce the row through DRAM
                    # into [128,8] partition-major layout (DRAM APs are linear
                    # so the transpose balances), reciprocal there (free size
                    # 8, ~0.2us), and bounce back to row layout for the
                    # broadcast.  No tensor-engine involvement.
                    # For the very last head-pair there is no later tensor work
                    # to hide the 4-hop DMA latency behind, so the direct
                    # (slow but short-chain) reciprocal wins there.
                    if last:
                        for s in range(2):
                            nc.vector.reciprocal(
                                recrow4[0:1, 1024 * p + s * QW:1024 * p + (s + 1) * QW],
                                ocp[64:65, s * QW:(s + 1) * QW],
                            )
                    else:
                        nc.sync.dma_start(out=rsum_d[p:p + 1, :], in_=ocp[64:65, :])
                        nc.sync.dma_start(
                            out=rsT8[:],
                            in_=rsum_d[p:p + 1, :].rearrange("p (c q) -> p q c", c=8),
                        )
                        nc.vector.reciprocal(rec8[:], rsT8[:])
                        nc.sync.dma_start(out=rec_d[p:p + 1, :], in_=rec8[:])
                        nc.sync.dma_start(
                            out=recrow4[0:1, 1024 * p:1024 * (p + 1)],
                            in_=rec_d[p:p + 1, :].rearrange("p (q c) -> p c q", c=8),
                        )
                    for s in range(2):
                        bc = work.tile([64, QW], F32, tag="bc", bufs=3)
                        nc.gpsimd.partition_broadcast(
                            bc[:],
                            recrow4[0:1, 1024 * p + s * QW:1024 * p + (s + 1) * QW],
                            channels=64,
                        )
                        nc.vector.tensor_mul(
                            ot[s * 64:(s + 1) * 64, p, jr],
                            ocp[0:64, s * QW:(s + 1) * QW],
                            bc[:],
                        )

            def outproj_gen(j):
                jr = slice(j * QW, (j + 1) * QW)
                for n in range(8):
                    yps = psum.tile([128, QW], F32, tag="small", bufs=2)
                    for c in range(4):
                        nc.tensor.matmul(
                            yps[:],
                            wo[:, c, n * 128:(n + 1) * 128],
                            ot[:, c, jr],
                            start=(c == 0), stop=(c == 3),
                        )
                        yield
                    ysb = work.tile([128, QW], BF16, tag="ysb", bufs=3)
                    nc.vector.tensor_scalar_add(ysb[:], yps[:], bias[:, n:n + 1])
                    nc.sync.dma_start(
                        out=yt_d[n * 128:(n + 1) * 128, jr], in_=ysb[:]
                    )
                    yield

            proj(0)
            for j in range(NJ):
                # During attn(j): fillers are outproj(j-1) units, then
                # proj(j+1) units queued per head-pair.  Everything proj(j+1)
                # must be fully emitted before attn(j+1) begins (tensor engine
                # executes in program order; a later filler cannot satisfy an
                # earlier instruction's dependency).
                for p in range(NP):
                    attn_p(j, p, last=(j == NJ - 1 and p == NP - 1))
                    if j + 1 < NJ:
                        pending.append(qk_gen(j + 1, p))
                if j + 1 < NJ:
                    pending.extend([v_gen(j + 1, sub) for sub in range(4)])
                drain_all()
                pending.append(outproj_gen(j))
            drain_all()

    nc.finalize()
    return nc


def _prep_inputs(x, Wq, Wk, Wv, Wo, bo):
    """Build the 8 per-core input maps (host-side layout prep only)."""
    import ml_dtypes

    scale = 1.0 / np.sqrt(np.float32(HD))
    kr = np.arange(128, dtype=np.float32)[:, None]
    qc = np.arange(128, dtype=np.float32)[None, :]
    tri = np.where(qc >= kr, np.float32(0.0), np.float32(-50.0))
    mneg = np.tile(tri, (1, 2)).astype(ml_dtypes.bfloat16)

    in_maps = []
    for c in range(NCORES):
        b, g = c // 2, c % 2
        hs = slice(g * 8, (g + 1) * 8)
        xt = np.ascontiguousarray(x[b].T).astype(ml_dtypes.bfloat16)
        wq = np.ascontiguousarray(Wq[hs].reshape(512, D).T * scale).astype(ml_dtypes.bfloat16)
        wk = np.ascontiguousarray(Wk[hs].reshape(512, D).T).astype(ml_dtypes.bfloat16)
        wv = np.ascontiguousarray(Wv[hs].reshape(512, D).T).astype(ml_dtypes.bfloat16)
        wo = np.ascontiguousarray(Wo[:, g * 512:(g + 1) * 512].T).astype(ml_dtypes.bfloat16)
        if g == 0:
            bias = np.ascontiguousarray(bo.reshape(8, 128).T)
        else:
            bias = np.zeros((128, 8), np.float32)
        in_maps.append(
            {"xt": xt, "wq": wq, "wk": wk, "wv": wv, "wo": wo, "bias": bias, "mask": mneg}
        )
    return in_maps


def _run(inputs, trace=False, trace_cores=None):
    from concourse.bass_utils import run_bass_kernel_spmd

    if "nc" not in _CACHE:
        _CACHE["nc"] = _build_nc()
    nc = _CACHE["nc"]
    in_maps = _prep_inputs(
        inputs["x"], inputs["Wq"], inputs["Wk"], inputs["Wv"], inputs["Wo"], inputs["bo"]
    )
    r = run_bass_kernel_spmd(
        nc, in_maps, list(range(NCORES)), trace=trace, trace_cores=trace_cores
    )
    y = np.empty((B, T, D), np.float32)
    for b in range(B):
        yt = np.asarray(r.results[2 * b]["yt"], dtype=np.float32) + np.asarray(
            r.results[2 * b + 1]["yt"], dtype=np.float32
        )
        y[b] = yt.T
    return y, r


def kernel(**inputs):
    y, _ = _run(inputs, trace=False)
    return y
